# revision 5
# baseline (speedup 1.0000x reference)
"""BDH layer (sparse-attention GLA block) on 8 Trainium2 NeuronCores.

Sharding: data-parallel over B (2) x tensor-parallel over heads (4).
Core c handles batch c//4, head c%4. Each core computes its head's partial
decoder output yMLP; a 4-core AllReduce per batch group sums them and every
core finishes the final norms. Host gathers y from core 0 (b=0) / core 4 (b=1).

Self-contained: hardcodes the problem shapes (B=2, T=1024, D=256, NH=4,
N=4096), builds/compiles the Bass program once per process, and runs it via
run_bass_kernel_spmd on cores 0-7.
"""

import math
import numpy as np

import concourse.bass as bass
import concourse.mybir as mybir
import concourse.tile as tile
from concourse import bacc
from concourse.bass_utils import run_bass_kernel_spmd

F32 = mybir.dt.float32
F32R = mybir.dt.float32r
BF16 = mybir.dt.bfloat16
AF = mybir.ActivationFunctionType
ALU = mybir.AluOpType

# ---- problem constants ----
B, T, D, NH, N = 2, 1024, 256, 4, 4096
NK = N                      # per-head key width (one head per core)
C = 256                     # GLA chunk length used by this kernel (exact math)
ROPE_BASE = float(2 ** 18)
SCALE_BASE = 512.0
GATE_DIV = 1024.0
EPS = 1e-5
KT = NK // 128              # 32 k-tiles
NS = T // C                 # 4 sweeps
CC = C // 128               # 2
DT = D // 128               # 2
TT = T // 128               # 8
SCALE = N ** -0.5
N_CORES = 8
GROUPS = [[0, 1, 2, 3], [4, 5, 6, 7]]


def _rope_tables():
    d = 256
    inv_freq = 1.0 / (ROPE_BASE ** (np.arange(0, d, 2, dtype=np.float64) / d))
    t = np.arange(T, dtype=np.float64)
    freqs = t[:, None] * inv_freq[None, :]
    scale = (np.arange(0, d, 2, dtype=np.float64) + 0.4 * d) / (1.4 * d)
    power = (t - float(T // 2)) / SCALE_BASE
    sc = scale[None, :] ** power[:, None]
    cos = (np.cos(freqs) * sc).astype(np.float32)
    sin = (np.sin(freqs) * sc).astype(np.float32)
    return np.ascontiguousarray(cos.T), np.ascontiguousarray(sin.T)


def _build():
    nc = bacc.Bacc("TRN2", target_bir_lowering=False, debug=False,
                   num_devices=N_CORES)

    wenc = nc.dram_tensor("wenc", [DT, KT, 128, 128], F32R, kind="ExternalInput")
    wgate = nc.dram_tensor("wgate", [DT, KT, 128, 128], F32R, kind="ExternalInput")
    wencv = nc.dram_tensor("wencv", [DT, KT, 128, 128], F32R, kind="ExternalInput")
    wdec = nc.dram_tensor("wdec", [KT, 128, D], F32R, kind="ExternalInput")
    xt = nc.dram_tensor("xt", [DT, 128, T], F32R, kind="ExternalInput")
    xv = nc.dram_tensor("xv", [TT, 128, D], F32R, kind="ExternalInput")
    cos_t = nc.dram_tensor("cos_t", [128, T], F32, kind="ExternalInput")
    sin_t = nc.dram_tensor("sin_t", [128, T], F32, kind="ExternalInput")
    triu = nc.dram_tensor("triu", [CC, 128, C], F32R, kind="ExternalInput")
    ident = nc.dram_tensor("ident", [128, 128], F32R, kind="ExternalInput")
    y_out = nc.dram_tensor("y", [TT, 128, D], F32, kind="ExternalOutput")

    ar_in = nc.dram_tensor("ar_in", [TT, 128, D], F32)
    ar_out = nc.dram_tensor("ar_out", [TT, 128, D], F32)

    ln_s = math.log(SCALE)
    relu_gate_scale = 1.0 / math.sqrt(GATE_DIV)

    with tile.TileContext(nc) as tc:
        with (
            tc.tile_pool(name="persist", bufs=1) as p_per,
            tc.tile_pool(name="wstream", bufs=4) as p_w,
            tc.tile_pool(name="wdecs", bufs=4) as p_wd,
            tc.tile_pool(name="tran", bufs=2) as p_t,
            tc.tile_pool(name="qk", bufs=6) as p_qk,
            tc.tile_pool(name="xsp", bufs=KT + 2) as p_xsp,
            tc.tile_pool(name="hpool", bufs=KT) as p_h,
            tc.tile_pool(name="small", bufs=2) as p_s,
            tc.tile_pool(name="ps_eg", bufs=2, space="PSUM") as ps_eg,
            tc.tile_pool(name="ps_at", bufs=1, space="PSUM") as ps_at,
            tc.tile_pool(name="ps_o", bufs=1, space="PSUM") as ps_o,
            tc.tile_pool(name="ps_upd", bufs=1, space="PSUM") as ps_upd,
        ):
            xt_sb = p_per.tile([128, DT * T], F32R, tag="xt")
            for d_ in range(DT):
                nc.sync.dma_start(xt_sb[:, d_ * T:(d_ + 1) * T], xt[d_])
            xv_sb = p_per.tile([128, TT * D], F32R, tag="xv")
            for t_ in range(TT):
                nc.sync.dma_start(xv_sb[:, t_ * D:(t_ + 1) * D], xv[t_])
            cos_sb = p_per.tile([128, T], F32, tag="cos")
            nc.sync.dma_start(cos_sb[:], cos_t[:])
            sin_sb = p_per.tile([128, T], F32, tag="sin")
            nc.sync.dma_start(sin_sb[:], sin_t[:])
            triu_sb = p_per.tile([128, CC * C], F32R, tag="triu")
            for j in range(CC):
                nc.sync.dma_start(triu_sb[:, j * C:(j + 1) * C], triu[j])
            id_sb = p_per.tile([128, 128], F32R, tag="ident")
            nc.sync.dma_start(id_sb[:], ident[:])
            id_f32 = id_sb[:].bitcast(F32)

            yMLP = p_per.tile([128, TT * D], F32, tag="ymlp")
            lns_c = p_per.tile([128, 1], F32, tag="lns")
            nc.gpsimd.memset(lns_c[:], ln_s)
            eps_c = p_per.tile([128, 1], F32, tag="epsc")
            nc.gpsimd.memset(eps_c[:], EPS)

            h_tiles = [p_h.tile([128, D], F32R, tag="h", name=f"h{i}")
                       for i in range(KT)]

            def xtile(d_, s):
                return xt_sb[:, d_ * T + s * C: d_ * T + (s + 1) * C]

            def vtile(s, j):
                t_ = s * CC + j
                return xv_sb[:, t_ * D:(t_ + 1) * D]

            for s in range(NS):
                csl = slice(s * C, (s + 1) * C)
                db_all = p_s.tile([128, KT], F32, tag="db", name=f"db{s}")
                at_ps = [ps_at.tile([128, C], F32, tag=f"at{j}", name=f"at{j}_{s}")
                         for j in range(CC)]
                o_ps = [ps_o.tile([128, D], F32, tag=f"o{g}", name=f"o{g}_{s}")
                        for g in range(CC)]

                xsp_tiles = [None] * KT
                qg_tiles = [None] * KT
                kg_tiles = [None] * KT

                for j2 in range(KT // 2):
                    pair = (2 * j2, 2 * j2 + 1)
                    eb_t, enb_t = {}, {}
                    for u, kt in enumerate(pair):
                        we = [p_w.tile([128, 128], F32R, tag=f"wenc{d_}",
                                       name=f"we{d_}_{s}_{kt}") for d_ in range(DT)]
                        for d_ in range(DT):
                            nc.sync.dma_start(we[d_][:], wenc[d_, kt])
                        eps_t = ps_eg.tile([128, C], F32, tag="eg", name=f"pe{s}_{kt}")
                        for d_ in range(DT):
                            nc.tensor.matmul(eps_t[:], we[d_][:], xtile(d_, s),
                                             start=(d_ == 0), stop=(d_ == DT - 1))
                        raw = p_t.tile([128, C], F32, tag="raw", name=f"raw{s}_{kt}")
                        nc.scalar.activation(raw[:], eps_t[:], AF.Copy)
                        xsp = p_xsp.tile([128, C], F32, tag="xsp",
                                         name=f"xsp{s}_{kt}")
                        nc.vector.scalar_tensor_tensor(
                            xsp[:], raw[:], 0.0, raw[:], ALU.max, ALU.mult)
                        xsp_tiles[kt] = xsp

                        wg = [p_w.tile([128, 128], F32R, tag=f"wgate{d_}",
                                       name=f"wg{d_}_{s}_{kt}") for d_ in range(DT)]
                        for d_ in range(DT):
                            nc.sync.dma_start(wg[d_][:], wgate[d_, kt])
                        gps_t = ps_eg.tile([128, C], F32, tag="eg", name=f"pg{s}_{kt}")
                        for d_ in range(DT):
                            nc.tensor.matmul(gps_t[:], wg[d_][:], xtile(d_, s),
                                             start=(d_ == 0), stop=(d_ == DT - 1))
                        rg = p_t.tile([128, C], F32, tag="rg", name=f"rg{s}_{kt}")
                        nc.scalar.activation(rg[:], gps_t[:], AF.Relu,
                                             scale=relu_gate_scale)
                        g2 = p_t.tile([128, C], F32, tag="g2", name=f"g2{s}_{kt}")
                        nc.gpsimd.tensor_tensor(g2[:], rg[:], rg[:], ALU.mult)
                        bneg = p_t.tile([128, C], F32, tag="bneg",
                                        name=f"bn{s}_{kt}")
                        nc.vector.tensor_tensor_scan(
                            bneg[:], g2[:], g2[:], 0.0, ALU.add, ALU.bypass)
                        eb = p_t.tile([128, C], F32R, tag="eb", name=f"eb{s}_{kt}", bufs=4)
                        nc.scalar.activation(eb[:], bneg[:], AF.Exp,
                                             scale=-1.0, bias=lns_c[:])
                        enb = p_t.tile([128, C], F32R, tag="enb", name=f"enb{s}_{kt}", bufs=4)
                        nc.scalar.activation(enb[:], bneg[:], AF.Exp)
                        nc.scalar.activation(db_all[:, kt:kt + 1],
                                             bneg[:, C - 1:C], AF.Exp, scale=-1.0)
                        eb_t[u], enb_t[u] = eb, enb

                    x0, x1 = xsp_tiles[pair[0]], xsp_tiles[pair[1]]
                    cos_s, sin_s = cos_sb[:, csl], sin_sb[:, csl]
                    m1 = p_t.tile([128, C], F32, tag="m1", name=f"m1_{s}_{j2}")
                    nc.vector.tensor_tensor(m1[:], x0[:], cos_s, ALU.mult)
                    m2 = p_t.tile([128, C], F32, tag="m2", name=f"m2_{s}_{j2}")
                    nc.gpsimd.tensor_tensor(m2[:], x1[:], sin_s, ALU.mult)
                    rot0 = p_t.tile([128, C], F32, tag="rot0", name=f"r0_{s}_{j2}")
                    nc.vector.tensor_tensor(rot0[:], m1[:], m2[:], ALU.subtract)
                    m3 = p_t.tile([128, C], F32, tag="m3", name=f"m3_{s}_{j2}")
                    nc.vector.tensor_tensor(m3[:], x0[:], sin_s, ALU.mult)
                    m4 = p_t.tile([128, C], F32, tag="m4", name=f"m4_{s}_{j2}")
                    nc.gpsimd.tensor_tensor(m4[:], x1[:], cos_s, ALU.mult)
                    rot1 = p_t.tile([128, C], F32, tag="rot1", name=f"r1_{s}_{j2}")
                    nc.vector.tensor_tensor(rot1[:], m3[:], m4[:], ALU.add)
                    for u, kt in enumerate(pair):
                        rot = rot0 if u == 0 else rot1
                        qg = p_qk.tile([128, C], F32R, tag="qg", name=f"qg{s}_{kt}")
                        nc.vector.tensor_tensor(qg[:], rot[:], eb_t[u][:], ALU.mult)
                        kg = p_qk.tile([128, C], F32R, tag="kg", name=f"kg{s}_{kt}")
                        nc.gpsimd.tensor_tensor(kg[:], rot[:], enb_t[u][:], ALU.mult)
                        qg_tiles[kt], kg_tiles[kt] = qg, kg

                    for u, kt in enumerate(pair):
                        qg, kg = qg_tiles[kt], kg_tiles[kt]
                        first, last = (kt == 0), (kt == KT - 1)
                        for j in range(CC):
                            nc.tensor.matmul(at_ps[j][:],
                                             kg[:, j * 128:(j + 1) * 128], qg[:],
                                             start=first, stop=last)
                        if s > 0:
                            for g in range(CC):
                                nc.tensor.matmul(
                                    o_ps[g][:], qg[:, g * 128:(g + 1) * 128],
                                    h_tiles[kt][:], start=first, stop=False)
                        if s < NS - 1:
                            tp = ps_upd.tile([128, C], F32, tag="tp",
                                             name=f"tp{s}_{kt}")
                            for j in range(CC):
                                nc.tensor.transpose(
                                    tp[:, j * 128:(j + 1) * 128],
                                    kg[:, j * 128:(j + 1) * 128].bitcast(F32),
                                    id_f32)
                            kgdbT = p_t.tile([128, C], F32R, tag="kgdbT",
                                             name=f"kgT{s}_{kt}")
                            nc.scalar.activation(kgdbT[:], tp[:], AF.Copy)
                            upd = ps_upd.tile([128, D], F32, tag="upd",
                                              name=f"up{s}_{kt}")
                            for j in range(CC):
                                nc.tensor.matmul(upd[:],
                                                 kgdbT[:, j * 128:(j + 1) * 128],
                                                 vtile(s, j), start=(j == 0),
                                                 stop=(s == 0 and j == CC - 1))
                            if s > 0:
                                nc.tensor.matmul(upd[:], id_sb[:], h_tiles[kt][:],
                                                 start=False, stop=True)
                            nc.vector.tensor_scalar_mul(
                                h_tiles[kt][:], upd[:], db_all[:, kt:kt + 1])

                at_sb = []
                for j in range(CC):
                    m = p_t.tile([128, C], F32R, tag=f"atsb{j}",
                                 name=f"atsb{j}_{s}")
                    nc.vector.tensor_tensor(
                        m[:], at_ps[j][:],
                        triu_sb[:, j * C:(j + 1) * C].bitcast(F32), ALU.mult)
                    at_sb.append(m)
                for g in range(CC):
                    for j in range(CC):
                        nc.tensor.matmul(o_ps[g][:],
                                         at_sb[j][:, g * 128:(g + 1) * 128],
                                         vtile(s, j),
                                         start=(s == 0 and j == 0),
                                         stop=(j == CC - 1))

                ykvt = p_s.tile([128, CC * C], F32R, tag="ykvt", name=f"ykvt{s}")
                for g in range(CC):
                    o_t = o_ps[g]
                    s1 = p_s.tile([128, 1], F32, tag="s1", name=f"s1_{s}_{g}")
                    nc.vector.tensor_reduce(s1[:], o_t[:], mybir.AxisListType.X,
                                            ALU.add)
                    sq = p_t.tile([128, D], F32, tag="sqscr", name=f"sq{s}_{g}")
                    s2 = p_s.tile([128, 1], F32, tag="s2", name=f"s2_{s}_{g}")
                    nc.scalar.activation(sq[:], o_t[:], AF.Square, accum_out=s2[:])
                    mean = p_s.tile([128, 1], F32, tag="mean", name=f"mn{s}_{g}")
                    nc.vector.tensor_scalar_mul(mean[:], s1[:], 1.0 / D)
                    var = p_s.tile([128, 1], F32, tag="var", name=f"vr{s}_{g}")
                    nc.vector.tensor_scalar_mul(var[:], s2[:], 1.0 / D)
                    msq = p_s.tile([128, 1], F32, tag="msq", name=f"mq{s}_{g}")
                    nc.vector.tensor_tensor(msq[:], mean[:], mean[:], ALU.mult)
                    nc.vector.tensor_tensor(var[:], var[:], msq[:], ALU.subtract)
                    std = p_s.tile([128, 1], F32, tag="std", name=f"sd{s}_{g}")
                    nc.scalar.activation(std[:], var[:], AF.Sqrt, bias=eps_c[:])
                    rstd = p_s.tile([128, 1], F32, tag="rstd", name=f"rs{s}_{g}")
                    nc.vector.reciprocal(rstd[:], std[:])
                    nbias = p_s.tile([128, 1], F32, tag="nbias", name=f"nb{s}_{g}")
                    nc.vector.tensor_tensor(nbias[:], mean[:], rstd[:], ALU.mult)
                    nc.vector.tensor_scalar_mul(nbias[:], nbias[:], -1.0)
                    ykv = p_t.tile([128, D], F32, tag="ykv", name=f"ykv{s}_{g}")
                    nc.scalar.activation(ykv[:], o_t[:], AF.Identity,
                                         scale=rstd[:], bias=nbias[:])
                    tp2 = ps_upd.tile([128, C], F32, tag="tp", name=f"tpy{s}_{g}")
                    for d_ in range(DT):
                        nc.tensor.transpose(tp2[:, d_ * 128:(d_ + 1) * 128],
                                            ykv[:, d_ * 128:(d_ + 1) * 128], id_f32)
                    for d_ in range(DT):
                        nc.scalar.activation(
                            ykvt[:, d_ * C + g * 128: d_ * C + (g + 1) * 128],
                            tp2[:, d_ * 128:(d_ + 1) * 128], AF.Copy)

                dec_ps = [ps_o.tile([128, D], F32, tag=f"o{g}", name=f"dc{g}_{s}")
                          for g in range(CC)]
                for kt in range(KT):
                    wv = [p_w.tile([128, 128], F32R, tag=f"wencv{d_}",
                                   name=f"wv{d_}_{s}_{kt}") for d_ in range(DT)]
                    for d_ in range(DT):
                        nc.sync.dma_start(wv[d_][:], wencv[d_, kt])
                    evps = ps_eg.tile([128, C], F32, tag="eg", name=f"pv{s}_{kt}")
                    for d_ in range(DT):
                        nc.tensor.matmul(evps[:], wv[d_][:],
                                         ykvt[:, d_ * C:(d_ + 1) * C],
                                         start=(d_ == 0), stop=(d_ == DT - 1))
                    t1 = p_t.tile([128, C], F32, tag="t1", name=f"t1_{s}_{kt}")
                    nc.vector.scalar_tensor_tensor(
                        t1[:], evps[:], 0.0, xsp_tiles[kt][:], ALU.max, ALU.mult)
                    xy = p_t.tile([128, C], F32R, tag="xy", name=f"xy{s}_{kt}")
                    nc.vector.scalar_tensor_tensor(
                        xy[:], evps[:], 0.0, t1[:], ALU.bypass, ALU.mult)
                    wd = p_wd.tile([128, D], F32R, tag="wdec", name=f"wd{s}_{kt}")
                    nc.sync.dma_start(wd[:], wdec[kt])
                    for g in range(CC):
                        nc.tensor.matmul(dec_ps[g][:],
                                         xy[:, g * 128:(g + 1) * 128], wd[:],
                                         start=(kt == 0), stop=(kt == KT - 1))
                for g in range(CC):
                    t_ = s * CC + g
                    nc.scalar.activation(yMLP[:, t_ * D:(t_ + 1) * D],
                                         dec_ps[g][:], AF.Copy)

            for t_ in range(TT):
                nc.sync.dma_start(ar_in[t_], yMLP[:, t_ * D:(t_ + 1) * D])
            nc.gpsimd.collective_compute(
                "AllReduce", ALU.add, replica_groups=GROUPS,
                ins=[ar_in[:]], outs=[ar_out[:]])
            ym2 = yMLP
            for t_ in range(TT):
                nc.sync.dma_start(ym2[:, t_ * D:(t_ + 1) * D], ar_out[t_])

            for t_ in range(TT):
                ym = ym2[:, t_ * D:(t_ + 1) * D]
                s1 = p_s.tile([128, 1], F32, tag="s1", name=f"fs1_{t_}")
                nc.vector.tensor_reduce(s1[:], ym, mybir.AxisListType.X, ALU.add)
                sq = p_t.tile([128, D], F32, tag="sqscr", name=f"fsq{t_}")
                s2 = p_s.tile([128, 1], F32, tag="s2", name=f"fs2_{t_}")
                nc.scalar.activation(sq[:], ym, AF.Square, accum_out=s2[:])
                mean = p_s.tile([128, 1], F32, tag="mean", name=f"fmn{t_}")
                nc.vector.tensor_scalar_mul(mean[:], s1[:], 1.0 / D)
                var = p_s.tile([128, 1], F32, tag="var", name=f"fvr{t_}")
                nc.vector.tensor_scalar_mul(var[:], s2[:], 1.0 / D)
                msq = p_s.tile([128, 1], F32, tag="msq", name=f"fmq{t_}")
                nc.vector.tensor_tensor(msq[:], mean[:], mean[:], ALU.mult)
                nc.vector.tensor_tensor(var[:], var[:], msq[:], ALU.subtract)
                std = p_s.tile([128, 1], F32, tag="std", name=f"fsd{t_}")
                nc.scalar.activation(std[:], var[:], AF.Sqrt, bias=eps_c[:])
                rstd = p_s.tile([128, 1], F32, tag="rstd", name=f"frs{t_}")
                nc.vector.reciprocal(rstd[:], std[:])
                nbias = p_s.tile([128, 1], F32, tag="nbias", name=f"fnb{t_}")
                nc.vector.tensor_tensor(nbias[:], mean[:], rstd[:], ALU.mult)
                nc.vector.tensor_scalar_mul(nbias[:], nbias[:], -1.0)
                ln = p_t.tile([128, D], F32, tag="ln", name=f"fln{t_}")
                nc.scalar.activation(ln[:], ym, AF.Identity,
                                     scale=rstd[:], bias=nbias[:])
                z = p_t.tile([128, D], F32, tag="z", name=f"fz{t_}")
                nc.vector.tensor_tensor(
                    z[:], ln[:], xv_sb[:, t_ * D:(t_ + 1) * D].bitcast(F32),
                    ALU.add)
                sq2 = p_t.tile([128, D], F32, tag="sqscr2", name=f"fq2{t_}")
                ms = p_s.tile([128, 1], F32, tag="ms", name=f"fms{t_}")
                nc.scalar.activation(sq2[:], z[:], AF.Square, accum_out=ms[:])
                nc.vector.tensor_scalar_mul(ms[:], ms[:], 1.0 / D)
                rms = p_s.tile([128, 1], F32, tag="rms", name=f"frm{t_}")
                nc.scalar.activation(rms[:], ms[:], AF.Sqrt, bias=eps_c[:])
                rr = p_s.tile([128, 1], F32, tag="rr", name=f"frr{t_}")
                nc.vector.reciprocal(rr[:], rms[:])
                yo = p_t.tile([128, D], F32, tag="yo", name=f"fy{t_}")
                nc.scalar.activation(yo[:], z[:], AF.Copy, scale=rr[:])
                nc.sync.dma_start(y_out[t_], yo[:])

    nc.compile()
    return nc


def _tile_w(W):
    return np.ascontiguousarray(
        np.asarray(W, dtype=np.float32).reshape(DT, 128, KT, 128)
        .transpose(0, 2, 1, 3))


_STATE = {}


def _get_nc():
    if "nc" not in _STATE:
        _STATE["nc"] = _build()
    return _STATE["nc"]


def _core_in_map(x_b, W_enc_h, W_gate_h, W_encv_h, W_dec_h, consts):
    cos_t, sin_t, triu, ident = consts
    xT = np.ascontiguousarray(x_b.T)
    return {
        "wenc": _tile_w(W_enc_h),
        "wgate": _tile_w(W_gate_h),
        "wencv": _tile_w(W_encv_h),
        "wdec": np.ascontiguousarray(
            np.asarray(W_dec_h, dtype=np.float32).reshape(KT, 128, D)),
        "xt": np.ascontiguousarray(xT.reshape(DT, 128, T)),
        "xv": np.ascontiguousarray(x_b.reshape(TT, 128, D)),
        "cos_t": cos_t, "sin_t": sin_t, "triu": triu, "ident": ident,
    }


def make_in_maps(x, W_enc, W_gate, W_dec, W_encv):
    x = np.asarray(x, dtype=np.float32)
    W_enc = np.asarray(W_enc, dtype=np.float32)
    W_gate = np.asarray(W_gate, dtype=np.float32)
    W_dec = np.asarray(W_dec, dtype=np.float32)
    W_encv = np.asarray(W_encv, dtype=np.float32)

    cos_t, sin_t = _rope_tables()
    triu = np.zeros((CC, 128, C), dtype=np.float32)
    for j in range(CC):
        for p in range(128):
            triu[j, p, j * 128 + p:] = 1.0
    ident = np.eye(128, dtype=np.float32)
    consts = (cos_t, sin_t, triu, ident)

    in_maps = []
    for c in range(N_CORES):
        b, h = c // 4, c % 4
        nsl = slice(h * N, (h + 1) * N)
        in_maps.append(_core_in_map(
            x[b], W_enc[:, nsl], W_gate[:, nsl], W_encv[h], W_dec[nsl, :],
            consts))
    return in_maps


def _get_runner():
    """Cached jitted SPMD executable mirroring bass2jax.run_bass_via_pjrt, so
    repeated kernel() calls skip re-tracing/recompiling."""
    if "runner" in _STATE:
        return _STATE["runner"]
    import jax
    import concourse.mybir as mb
    from concourse import bass2jax as b2j
    from jax.experimental.shard_map import shard_map
    from jax.sharding import Mesh, PartitionSpec

    nc = _get_nc()
    b2j.install_neuronx_cc_hook()
    partition_name = (nc.partition_id_tensor.name
                      if nc.partition_id_tensor else None)
    in_names, out_names, out_avals, zero_outs = [], [], [], []
    for alloc in nc.m.functions[0].allocations:
        if not isinstance(alloc, mb.MemoryLocationSet):
            continue
        name = alloc.memorylocations[0].name
        if alloc.kind == "ExternalInput":
            if name != partition_name:
                in_names.append(name)
        elif alloc.kind == "ExternalOutput":
            shape = tuple(alloc.tensor_shape)
            dtype = mb.dt.np(alloc.dtype)
            out_names.append(name)
            out_avals.append(jax.core.ShapedArray(shape, dtype))
            zero_outs.append(np.zeros(shape, dtype))
    n_params = len(in_names)
    all_names = in_names + out_names
    if partition_name is not None:
        all_names = all_names + [partition_name]
    donate = tuple(range(n_params, n_params + len(out_names)))

    def _body(*args):
        operands = list(args)
        if partition_name is not None:
            operands.append(b2j.partition_id_tensor())
        return tuple(b2j._bass_exec_p.bind(
            *operands,
            out_avals=tuple(out_avals),
            in_names=tuple(all_names),
            out_names=tuple(out_names),
            lowering_input_output_aliases=(),
            sim_require_finite=True,
            sim_require_nnan=True,
            nc=nc,
        ))

    devices = jax.devices()[:N_CORES]
    mesh = Mesh(np.asarray(devices), ("core",))
    in_specs = (PartitionSpec("core"),) * (n_params + len(out_names))
    out_specs = (PartitionSpec("core"),) * len(out_names)
    sharded = jax.jit(
        shard_map(_body, mesh=mesh, in_specs=in_specs, out_specs=out_specs,
                  check_rep=False),
        donate_argnums=donate, keep_unused=True)
    _STATE["runner"] = (sharded, in_names, out_names, out_avals, zero_outs, mesh)
    return _STATE["runner"]


def _concat_inputs(in_maps, in_names):
    return [np.concatenate([np.asarray(in_maps[c][nm]) for c in range(N_CORES)],
                           axis=0) for nm in in_names]


def _run(in_maps):
    sharded, in_names, out_names, out_avals, zero_outs, mesh = _get_runner()
    concat_in = _concat_inputs(in_maps, in_names)
    concat_zeros = [np.zeros((N_CORES * z.shape[0], *z.shape[1:]), z.dtype)
                    for z in zero_outs]
    out_arrs = sharded(*concat_in, *concat_zeros)
    return {name: np.asarray(out_arrs[i]).reshape(N_CORES, *out_avals[i].shape)
            for i, name in enumerate(out_names)}


def kernel(x, W_enc, W_gate, W_dec, W_encv):
    in_maps = make_in_maps(x, W_enc, W_gate, W_dec, W_encv)
    outs = _run(in_maps)
    y0 = outs["y"][0].reshape(T, D)
    y1 = outs["y"][4].reshape(T, D)
    return np.stack([y0, y1]).astype(np.float32)


def time_device_exec(np_inputs, iters=10):
    """Best wall-clock (ns) of the device execution with inputs pre-staged on
    device; excludes host prep and output conversion."""
    import time as _time
    import jax
    from jax.sharding import NamedSharding, PartitionSpec
    sharded, in_names, out_names, out_avals, zero_outs, mesh = _get_runner()
    in_maps = make_in_maps(**np_inputs)
    concat_in = _concat_inputs(in_maps, in_names)
    sh = NamedSharding(mesh, PartitionSpec("core"))
    dev_in = [jax.device_put(a, sh) for a in concat_in]
    for a in dev_in:
        a.block_until_ready()
    best = float("inf")
    for _ in range(iters):
        concat_zeros = [jax.device_put(
            np.zeros((N_CORES * z.shape[0], *z.shape[1:]), z.dtype), sh)
            for z in zero_outs]
        for a in concat_zeros:
            a.block_until_ready()
        t0 = _time.perf_counter()
        out = sharded(*dev_in, *concat_zeros)
        for o in out:
            o.block_until_ready()
        best = min(best, _time.perf_counter() - t0)
    return best * 1e9
